# revision 14
# baseline (speedup 1.0000x reference)
"""Trainium2 Bass kernel for a 4-layer transformer decoder (self-attn +
cross-attn + FFN, post-residual, exact GELU), distributed over 8 NeuronCores.

Sharding: data-parallel over batch (B=4 -> 4 core pairs); within a pair the
target sequence T=1024 is split in half (512 rows per core). Activations are
kept feature-major ("transposed", [D, T_half]) so every projection is a
single matmul chain with no transposes. Per layer each core computes its own
self-attn K/V shard (its 512 target rows) AND its cross-attn K/V shard (its
512 encoder rows), exchanged with its pair via two AllGathers (bf16). The
second AllGather is emitted after the SA attention so it does not block the
GpSimd queue (partition-broadcasts) during the SA softmax window.

All projections except the FFN run in fp8(e4m3) with the DoubleRow perf mode
(two 128-row contraction chunks per instruction): QKV / cross-KV use weights
pre-scaled by 32 (folded out through the softmax: exp-scale and the
ones-column of V), out-projections use unscaled fp8 weights. The FFN stays
bf16 (fp8 there fails the 2e-2 accuracy budget). Attention score/AV matmuls
run in bf16 with fp32 PSUM; score pairs (K=64) sit on distinct PE row-strips
(base partitions 0/64) and execute concurrently; the inner loop is
software-pipelined (scores[sc+1] before AV[sc]) with kT waves prefetched two
waves ahead. Softmax skips max-subtraction and row sums come free from the
scaled ones-column on V. Out-projections are emitted kc-major (all 8 output
psums live, contraction outer) so the in-order PE queue is not head-of-line
blocked by the last attention wave's normalization. The fp32 residual stream
stays in SBUF for all 4 layers. All biases in the reference are zero and are
folded out.

Self-contained: hardcodes all shapes; no file I/O, no sibling imports.
"""
import numpy as np
import ml_dtypes

import concourse.bass as bass
import concourse.mybir as mybir
import concourse.tile as tile
from concourse import bacc
from concourse import bass_utils

F32 = mybir.dt.float32
F32R = mybir.dt.float32r
BF16 = mybir.dt.bfloat16
F8 = mybir.dt.float8e4
DR = mybir.MatmulPerfMode.DoubleRow
EXP = mybir.ActivationFunctionType.Exp
GELU = mybir.ActivationFunctionType.Gelu

L, D, H, DK, HID = 4, 1024, 16, 64, 4096
B, T, S = 4, 1024, 1024
R = T // 2              # rows (target positions / encoder positions) per core
N_CORES = 8
DC = D // 128           # 8 feature chunks
SC = S // 128           # 8 key chunks
RC = R // 128           # 4 own-row chunks
NP = D // 256           # 4 contraction pairs for DoubleRow
KV_ELEMS = D * R + R * D        # kT flat + v flat per-core shard (bf16)
RG = [[0, 1], [2, 3], [4, 5], [6, 7]]
SW = 32.0               # fp8 weight pre-scale for QKV / cross-KV

_CACHE = {}


def _emit(nc, tc, pools, dram):
    (xp, xbp, x8p, e8p, w8p, w8kp, wbp, qp, kvp, kwp, vap, hp, accp,
     minip, minir, minib) = pools

    def dma(dst, src):
        nc.sync.dma_start(dst, src)

    def dma_s(dst, src):
        # store queue on the Activation HWDGE: keeps collective-input stores
        # out of the weight-prefetch queue (ACT is idle during the KV phases)
        nc.scalar.dma_start(dst, src)

    def r2(t8):
        return t8[:].rearrange("p (ko r) -> p ko r", ko=2)

    def proj8(w8_ap, rhs8, noc, col0, consume, ppool, kcmajor=False):
        """fp8 DoubleRow projection: psum[oc] = sum_p
        W[p-pair, col0+oc*128:+128].T @ rhs8[pair p]; consume(oc, psum).

        w8_ap: [NP, 128, 2*dout] pair-packed fp8 weights.
        rhs8: list of NP fp8 pair-tiles [128, 2*R] (separate tiles so the
        dependency tracker scopes each pair to its own writers).
        kcmajor: all `noc` psums live, pair as outer loop (tail-hiding)."""
        rv = [r2(t) for t in rhs8]
        if kcmajor:
            psums = []
            for oc in range(noc):
                ps = ppool.tile([128, R], F32, tag="pj", name=f"p8k{oc}")
                psums.append(ps)
            for p in range(NP):
                ws = w8kp.tile([128, 2 * noc * 128], F8, tag="ws8k")
                wv = ws[:].rearrange("k (ko m) -> k ko m", ko=2)
                c0 = col0
                dma(wv, w8_ap[p].rearrange("k (ko m) -> k ko m", ko=2)
                    [:, :, c0:c0 + noc * 128])
                for oc in range(noc):
                    nc.tensor.matmul(
                        psums[oc][:], wv[:, :, oc * 128:(oc + 1) * 128],
                        rv[p],
                        start=(p == 0), stop=(p == NP - 1), perf_mode=DR)
            for oc in range(noc):
                consume(oc, psums[oc])
        else:
            for g0 in range(0, noc, 4):
                psums = []
                for _ in range(4):
                    ps = ppool.tile([128, R], F32, tag="pj")
                    psums.append(ps)
                for p in range(NP):
                    ws = w8p.tile([128, 1024], F8, tag="ws8")
                    wv = ws[:].rearrange("k (ko m) -> k ko m", ko=2)
                    c0 = col0 + g0 * 128
                    dma(wv, w8_ap[p].rearrange("k (ko m) -> k ko m", ko=2)
                        [:, :, c0:c0 + 512])
                    for j in range(4):
                        nc.tensor.matmul(
                            psums[j][:], wv[:, :, j * 128:(j + 1) * 128],
                            rv[p],
                            start=(p == 0), stop=(p == NP - 1), perf_mode=DR)
                for j in range(4):
                    consume(g0 + j, psums[j])

    def attention(q_tiles, kT_of_wave, va_tiles, spool, avpool, escale, avT8):
        """Feature-major attention, software-pipelined over (wave, sc).
        Writes normalized outputs into avT8 (4 fp8 pair-tiles [128, 2R])."""
        NW = H // 2
        kw = [None] * NW
        avs = [None] * NW
        pend = None  # (w, sc, p_t)

        def ensure_kw(w):
            if w < NW and kw[w] is None:
                kw[w] = kT_of_wave(w)

        def flush(p):
            w_, sc_, pt_ = p
            for hi in range(2):
                nc.tensor.matmul(
                    avs[w_][hi][0:65, :],
                    va_tiles[sc_][:, (2 * w_ + hi) * 65:(2 * w_ + hi + 1) * 65],
                    pt_[:, hi * R:(hi + 1) * R],
                    start=(sc_ == 0), stop=(sc_ == SC - 1))
            if sc_ == SC - 1:
                for hi in range(2):
                    rec = minir.tile([1, R], F32, tag="rec")
                    nc.vector.reciprocal(rec[:], avs[w_][hi][64:65, :])
                    bc = minib.tile([64, R], F32, tag="bc")
                    nc.gpsimd.partition_broadcast(bc[:], rec[:])
                    nc.vector.tensor_mul(
                        avT8[w_ // 2][hi * 64:(hi + 1) * 64,
                                      (w_ % 2) * R:(w_ % 2 + 1) * R],
                        avs[w_][hi][0:64, :], bc[:])

        ensure_kw(0)
        ensure_kw(1)
        for w in range(NW):
            ensure_kw(w + 2)
            avs[w] = [avpool.tile([128, R], F32, tag="av", name=f"av{w}_{i}")
                      for i in range(2)]
            for sc in range(SC):
                slab = spool.tile([128, 2 * R], F32, tag="sc")
                p_t = minip.tile([128, 2 * R], BF16, tag="p")
                for hi in range(2):
                    nc.tensor.matmul(
                        slab[:, hi * R:(hi + 1) * R],
                        kw[w][hi * 64:(hi + 1) * 64, sc * 128:(sc + 1) * 128],
                        q_tiles[w][hi * 64:(hi + 1) * 64, :],
                        start=True, stop=True)
                if pend is not None:
                    flush(pend)
                nc.scalar.activation(p_t[:], slab[:], EXP, scale=escale)
                pend = (w, sc, p_t)
        flush(pend)

    def phase_kv8(li, rhs8, w8_ap, k_col0, v_col0, ccin, pname):
        """fp8 DoubleRow K^T [D, 512] + V [512, D] shard projections, stored
        to ccin (bf16, x32-scaled values) for the pair AllGather."""
        rv = [r2(t) for t in rhs8]
        with tc.tile_pool(name=pname, bufs=8, space="PSUM") as pA:
            def mk_k(oc, ps):
                kt = kvp.tile([128, R], BF16, tag="ko")
                nc.vector.tensor_copy(kt[:], ps[:])
                dma_s(ccin[oc * 128 * R:(oc + 1) * 128 * R]
                      .rearrange("(p s) -> p s", p=128), kt[:])
            proj8(w8_ap, rhs8, DC, k_col0, mk_k, pA)

            for vc in range(2):
                psv = []
                for _ in range(RC):
                    ps = pA.tile([128, 512], F32, tag="pj")
                    psv.append(ps)
                for p in range(NP):
                    ws = w8p.tile([128, 1024], F8, tag="ws8")
                    wv = ws[:].rearrange("k (ko m) -> k ko m", ko=2)
                    dma(wv, w8_ap[p].rearrange("k (ko m) -> k ko m", ko=2)
                        [:, :, v_col0 + vc * 512:v_col0 + (vc + 1) * 512])
                    for t_ in range(RC):
                        nc.tensor.matmul(
                            psv[t_][:],
                            rv[p][:, :, t_ * 128:(t_ + 1) * 128],
                            wv,
                            start=(p == 0), stop=(p == NP - 1), perf_mode=DR)
                for t_ in range(RC):
                    vt = kvp.tile([128, 512], BF16, tag="vo")
                    nc.vector.tensor_copy(vt[:], psv[t_][:])
                    rbase = D * R + t_ * 128 * D
                    dst = (ccin[rbase:rbase + 128 * D]
                           .rearrange("(p f) -> p f", f=D)
                           [:, vc * 512:(vc + 1) * 512])
                    dma_s(dst, vt[:])

    def phase_q8(li, rhs8, w8_ap, pname):
        qT = [None] * DC
        with tc.tile_pool(name=pname, bufs=8, space="PSUM") as pQ:
            def mk_q(oc, ps):
                t = qp.tile([128, R], BF16, tag="q")
                nc.vector.tensor_copy(t[:], ps[:])
                qT[oc] = t
            proj8(w8_ap, rhs8, DC, 0, mk_q, pQ)
        return qT

    def phase_attn(li, qT, ccout, escale, pname):
        """Attention over a gathered KV buffer: blk0 = pair-even rows,
        blk1 = pair-odd rows. Returns fp8 avT8 [128, 8, R]."""
        va = []
        for sc in range(SC):
            sav = vap.tile([128, H * 65], BF16, tag="sav")
            sav3 = sav[:].rearrange("p (h w) -> p h w", w=65)
            nc.gpsimd.memset(sav3[:, :, 64:65], SW)
            blk = sc // 4
            rbase = blk * KV_ELEMS + D * R + (sc % 4) * 128 * D
            src = (ccout[rbase:rbase + 128 * D]
                   .rearrange("(p f) -> p f", f=D)
                   .rearrange("p (h w) -> p h w", w=DK))
            dma(sav3[:, :, 0:DK], src)
            va.append(sav)

        def kT_wave(w):
            kw = kwp.tile([128, S], BF16, tag="kw")
            for blk in range(2):
                base = blk * KV_ELEMS + w * 128 * R
                dma(kw[:, blk * R:(blk + 1) * R],
                    ccout[base:base + 128 * R]
                    .rearrange("(p s) -> p s", p=128))
            return kw

        avT8 = [x8p.tile([128, 2 * R], F8, tag="av8",
                         name=f"av8{pname}{li}_{p}") for p in range(NP)]
        with (
            tc.tile_pool(name=f"ps{pname}{li}", bufs=2, space="PSUM") as sD,
            tc.tile_pool(name=f"pa{pname}{li}", bufs=4, space="PSUM") as aD,
        ):
            attention(qT, kT_wave, va, sD, aD, escale, avT8)
        return avT8

    def phase_proj_res8(li, name, w8_ap, rhs8, res_tiles, shadows):
        """x_out = W.T @ rhs + res via fp8 DoubleRow, kc-major. `shadows`
        selects extra per-chunk copies: "f8" (fp8 [128,8,R]) or "bf" (bf16
        tiles). Returns (x_tiles, shadow)."""
        xo = [None] * DC
        xb = [None] * DC
        x8 = None
        if shadows == "f8":
            x8 = [x8p.tile([128, 2 * R], F8, tag="x8",
                           name=f"x8{name}{li}_{p}") for p in range(NP)]
        with tc.tile_pool(name=f"ps{name}{li}", bufs=8, space="PSUM") as pp:
            def mk(oc, ps):
                t = xp.tile([128, R], F32R, tag="x")
                nc.vector.tensor_add(t[:], ps[:],
                                     res_tiles[oc][:].bitcast(F32))
                xo[oc] = t
                if shadows == "f8":
                    nc.vector.tensor_copy(
                        x8[oc // 2][:, (oc % 2) * R:(oc % 2 + 1) * R],
                        t[:].bitcast(F32))
                elif shadows == "bf":
                    tb = xbp.tile([128, R], BF16, tag="x2b")
                    nc.vector.tensor_copy(tb[:], t[:].bitcast(F32))
                    xb[oc] = tb
            proj8(w8_ap, rhs8, DC, 0, mk, pp, kcmajor=True)
        return xo, (x8 if shadows == "f8" else xb)

    def phase_ffn(li, wf1, wf2, x2, x2b):
        """bf16 FFN; produces x3 (f32r) + fp8 shadow for the next layer."""
        acc = [None] * DC
        with tc.tile_pool(name=f"psI{li}", bufs=8, space="PSUM") as pI:
            for qtr in range(4):
                hq = [None] * DC
                def mk_h(oc, ps, hq=hq):
                    t = hp.tile([128, R], BF16, tag="h")
                    nc.scalar.activation(t[:], ps[:], GELU)
                    hq[oc] = t
                proj_bf(wf1, x2b, qtr * D, mk_h, pI)
                wf2q = wf2[qtr * D:(qtr + 1) * D, :]
                def mk_acc(oc, ps, qtr=qtr):
                    if qtr == 0:
                        t = accp.tile([128, R], F32, tag="acc")
                        nc.vector.tensor_add(t[:], ps[:],
                                             x2[oc][:].bitcast(F32))
                        acc[oc] = t
                    else:
                        nc.vector.tensor_add(acc[oc][:], ps[:], acc[oc][:])
                proj_bf(wf2q, hq, 0, mk_acc, pI)
        x3 = [None] * DC
        x8n = [x8p.tile([128, 2 * R], F8, tag="x8",
                        name=f"x8n{li}_{p}") for p in range(NP)]
        for oc in range(DC):
            xt3 = xp.tile([128, R], F32R, tag="x")
            nc.vector.tensor_copy(xt3[:], acc[oc][:])
            x3[oc] = xt3
            nc.vector.tensor_copy(
                x8n[oc // 2][:, (oc % 2) * R:(oc % 2 + 1) * R], acc[oc][:])
        return x3, x8n

    def proj_bf(w_ap, rhs_tiles, col0, consume, ppool):
        """bf16 transposed-mode projection (FFN), groups of 4 out chunks."""
        kcn = len(rhs_tiles)
        for g0 in range(0, DC, 4):
            psums = []
            for _ in range(4):
                ps = ppool.tile([128, R], F32, tag="pj")
                psums.append(ps)
            for kc in range(kcn):
                ws = wbp.tile([128, 512], BF16, tag="wsb")
                c0 = col0 + g0 * 128
                dma(ws[:], w_ap[kc * 128:(kc + 1) * 128, c0:c0 + 512])
                for j in range(4):
                    nc.tensor.matmul(
                        psums[j][:], ws[:, j * 128:(j + 1) * 128],
                        rhs_tiles[kc][:],
                        start=(kc == 0), stop=(kc == kcn - 1))
            for j in range(4):
                consume(g0 + j, psums[j])

    # ---------------- main program ----------------
    (xT_d, xF8_d, encF8_d, w_qkv8, w_o8, w_cq8, w_cakv8, w_co8,
     w_ff1, w_ff2, out_d, cc_in, cc_out, cc2_in, cc2_out) = dram

    xT = []
    for ci in range(DC):
        xt = xp.tile([128, R], F32R, tag="x")
        dma(xt[:], xT_d.ap()[ci * 128:(ci + 1) * 128])
        xT.append(xt)
    xf8 = [x8p.tile([128, 2 * R], F8, tag="x8", name=f"x8in{p}")
           for p in range(NP)]
    ef8 = [e8p.tile([128, 2 * R], F8, tag="e8", name=f"e8in{p}")
           for p in range(NP)]
    for ci in range(DC):
        dma(xf8[ci // 2][:, (ci % 2) * R:(ci % 2 + 1) * R],
            xF8_d.ap()[ci * 128:(ci + 1) * 128])
        dma(ef8[ci // 2][:, (ci % 2) * R:(ci % 2 + 1) * R],
            encF8_d.ap()[ci * 128:(ci + 1) * 128])

    for li in range(L):
        ccin = cc_in[li].ap()
        ccout = cc_out[li].ap()
        ccin2 = cc2_in[li].ap()
        ccout2 = cc2_out[li].ap()
        phase_kv8(li, xf8, w_qkv8.ap()[li], D, 2 * D, ccin, f"psA{li}")
        nc.gpsimd.collective_compute(
            "AllGather", mybir.AluOpType.bypass, replica_groups=RG,
            ins=[ccin], outs=[ccout])
        phase_kv8(li, ef8, w_cakv8.ap()[li], 0, D, ccin2, f"psC{li}")
        qT = phase_q8(li, xf8, w_qkv8.ap()[li], f"psQ{li}")
        avT8 = phase_attn(li, qT, ccout, 0.125 / (SW * SW), "D")
        # AG2 after the SA attention: the collective blocks the GpSimd queue
        # until it completes, so emitting it earlier would stall the SA
        # softmax partition-broadcasts. Output first needed at CA attention.
        nc.gpsimd.collective_compute(
            "AllGather", mybir.AluOpType.bypass, replica_groups=RG,
            ins=[ccin2], outs=[ccout2])
        x1, x1f8 = phase_proj_res8(li, "E", w_o8.ap()[li], avT8, xT, "f8")
        caqT = phase_q8(li, x1f8, w_cq8.ap()[li], f"psF{li}")
        ca_avT8 = phase_attn(li, caqT, ccout2, 0.125 / SW, "G")
        x2, x2b = phase_proj_res8(li, "H", w_co8.ap()[li], ca_avT8, x1, "bf")
        xT, xf8 = phase_ffn(li, w_ff1.ap()[li], w_ff2.ap()[li], x2, x2b)

    for oc in range(DC):
        dma(out_d.ap()[oc * 128:(oc + 1) * 128], xT[oc][:].bitcast(F32))


def _build():
    nc = bacc.Bacc("TRN2", target_bir_lowering=False, debug=False,
                   num_devices=N_CORES)
    dram = (
        nc.dram_tensor("xT", [D, R], F32R, kind="ExternalInput"),
        nc.dram_tensor("xF8", [D, R], F8, kind="ExternalInput"),
        nc.dram_tensor("encF8", [D, R], F8, kind="ExternalInput"),
        nc.dram_tensor("w_qkv8", [L, NP, 128, 2 * 3 * D], F8,
                       kind="ExternalInput"),
        nc.dram_tensor("w_o8", [L, NP, 128, 2 * D], F8, kind="ExternalInput"),
        nc.dram_tensor("w_cq8", [L, NP, 128, 2 * D], F8, kind="ExternalInput"),
        nc.dram_tensor("w_cakv8", [L, NP, 128, 2 * 2 * D], F8,
                       kind="ExternalInput"),
        nc.dram_tensor("w_co8", [L, NP, 128, 2 * D], F8, kind="ExternalInput"),
        nc.dram_tensor("w_ff1", [L, D, HID], BF16, kind="ExternalInput"),
        nc.dram_tensor("w_ff2", [L, HID, D], BF16, kind="ExternalInput"),
        nc.dram_tensor("out", [D, R], F32, kind="ExternalOutput"),
        [nc.dram_tensor(f"cc_in{i}", [KV_ELEMS], BF16, kind="Internal")
         for i in range(L)],
        [nc.dram_tensor(f"cc_out{i}", [2 * KV_ELEMS], BF16, kind="Internal")
         for i in range(L)],
        [nc.dram_tensor(f"cc2_in{i}", [KV_ELEMS], BF16, kind="Internal")
         for i in range(L)],
        [nc.dram_tensor(f"cc2_out{i}", [2 * KV_ELEMS], BF16, kind="Internal")
         for i in range(L)],
    )
    with tile.TileContext(nc) as tc:
        with (
            tc.tile_pool(name="xp", bufs=13) as xp,      # f32r [128,R] resid
            tc.tile_pool(name="xbp", bufs=9) as xbp,     # bf16 [128,R] x2b
            tc.tile_pool(name="x8p", bufs=22) as x8p,    # fp8 [128,2R] pair shadows
            tc.tile_pool(name="e8p", bufs=4) as e8p,     # fp8 [128,2R] enc pairs
            tc.tile_pool(name="w8p", bufs=8) as w8p,     # fp8 [128,1024] slabs
            tc.tile_pool(name="w8kp", bufs=3) as w8kp,   # fp8 [128,2048] slabs
            tc.tile_pool(name="wbp", bufs=12) as wbp,    # bf16 [128,512] slabs
            tc.tile_pool(name="qp", bufs=8) as qp,       # bf16 [128,R] qT
            tc.tile_pool(name="kvp", bufs=3) as kvp,     # bf16 kv staging
            tc.tile_pool(name="kwp", bufs=3) as kwp,     # bf16 [128,S] kT wave
            tc.tile_pool(name="vap", bufs=8) as vap,     # bf16 [128,H*65] v_aug
            tc.tile_pool(name="hp", bufs=8) as hp,       # bf16 [128,R] ffn hid
            tc.tile_pool(name="accp", bufs=8) as accp,   # f32 [128,R] ffn acc
            tc.tile_pool(name="minip", bufs=4) as minip,  # bf16 p slabs
            tc.tile_pool(name="minir", bufs=2) as minir,  # rec rows
            tc.tile_pool(name="minib", bufs=2) as minib,  # bcast tiles
        ):
            pools = (xp, xbp, x8p, e8p, w8p, w8kp, wbp, qp, kvp, kwp, vap,
                     hp, accp, minip, minir, minib)
            _emit(nc, tc, pools, dram)
    nc.compile()
    return nc


def _get_nc():
    if "nc" not in _CACHE:
        _CACHE["nc"] = _build()
    return _CACHE["nc"]


def _pack8(w, scale):
    """[L, din, dout] float weights -> [L, din//256, 128, 2*dout] e4m3 in
    DoubleRow pair layout: out[l, p, ki, ko*dout+m] = w[l, p*256+ko*128+ki, m]."""
    w = np.asarray(w, np.float32) * scale
    Lw, din, dout = w.shape
    w8 = w.astype(ml_dtypes.float8_e4m3)
    return np.ascontiguousarray(
        w8.reshape(Lw, din // 256, 2, 128, dout).transpose(0, 1, 3, 2, 4)
        .reshape(Lw, din // 256, 128, 2 * dout))


def _prep_in_maps(inputs):
    tgt = np.asarray(inputs["tgt"], dtype=np.float32)
    enc_out = np.asarray(inputs["enc_out"], dtype=np.float32)
    shared = {
        "w_qkv8": _pack8(inputs["sa_qkv_w"], SW),
        "w_o8": _pack8(inputs["sa_out_w"], 1.0),
        "w_cq8": _pack8(inputs["ca_q_w"], 1.0),
        "w_cakv8": _pack8(inputs["ca_kv_w"], SW),
        "w_co8": _pack8(inputs["ca_out_w"], 1.0),
        "w_ff1": np.asarray(inputs["ff_w1"]).astype(ml_dtypes.bfloat16),
        "w_ff2": np.asarray(inputs["ff_w2"]).astype(ml_dtypes.bfloat16),
    }
    in_maps = []
    for c in range(N_CORES):
        b, hh = c // 2, c % 2
        xt = np.ascontiguousarray(tgt[b].T[:, hh * R:(hh + 1) * R])
        et = np.ascontiguousarray(enc_out[b].T[:, hh * R:(hh + 1) * R])
        m = {
            "xT": xt,
            "xF8": xt.astype(ml_dtypes.float8_e4m3),
            "encF8": et.astype(ml_dtypes.float8_e4m3),
        }
        m.update(shared)
        in_maps.append(m)
    return in_maps


def kernel(**inputs):
    nc = _get_nc()
    in_maps = _prep_in_maps(inputs)
    res = bass_utils.run_bass_kernel_spmd(nc, in_maps,
                                          core_ids=list(range(N_CORES)))
    out = np.empty((B, T, D), dtype=np.float32)
    for c in range(N_CORES):
        b, hh = c // 2, c % 2
        out[b, hh * R:(hh + 1) * R, :] = res.results[c]["out"].T
    return out
